# revision 14
# baseline (speedup 1.0000x reference)
"""EdgeConv2dDiff Trainium2 Bass kernel.

Reference computation (B=1, C=64, N=50000, K=16, COUT=64):
    e = concat([x_i, x_j - x_i], axis=channel)          # [B, 2C, N, K]
    y = relu(einsum("bcnk,oc->bonk", e, W) + b)          # [B, COUT, N, K]
    out = max(y, axis=K, keepdims=True)                  # [B, COUT, N, 1]

Algebraic restructuring used here:
    W1 @ x_i + W2 @ (x_j - x_i) == (W1 - W2) @ x_i + W2 @ x_j
so the folded weight  wT = [[(W1-W2).T], [W2.T]]  ([2C, COUT]) turns the
whole edge-feature construction into a single 128-contraction matmul over
a stacked input [x_i; x_j] ([2C, N*K]).  Also
    max_k(relu(z_k + b)) == relu(max_k(z_k) + b)
so the K-max runs on raw PSUM output and bias+relu touches 16x fewer
elements.

Sharding: data-parallel over nodes N across 8 cores (6250 nodes each),
no cross-core communication.

Per-core structure: the core's 6250 nodes are split into two halves of
3125; two input DMA streams (one per half) run in lockstep on the two
HWDGE rings (half A on the Sync ring, half B on the Scalar ring) so
descriptor generation for the two streams proceeds in parallel.  Each
PSUM tile takes a 32-node matmul from stream A on partitions 0:63 and
the matching 32-node matmul from stream B on partitions 64:127, so
every vector K-max reduce covers all 128 partitions.  Each chunk has
its own small output tile (bias+relu fused on DVE right after the
K-max, in-order, no cross-engine hop), so output flush DMAs never
WAR-serialize against the next chunk's compute.  Flushes ride the same
two HWDGE rings but are dispatched three chunks late, so their
relu-wait never parks a sequencer ahead of pending input dispatches.
The chunk schedule starts small (quick first bytes) and tapers at the
end so the post-last-DMA drain is one tiny chunk's K-max + relu +
flush.
"""

import sys

import numpy as np

for _p in ("/opt/trn_rl_repo",):
    if _p not in sys.path:
        sys.path.insert(0, _p)

B, C, N, K = 1, 64, 50000, 16
COUT = 64
NCORES = 8
NS = N // NCORES          # 6250 nodes per core
NSH = NS // 2             # 3125 nodes per half-stream
FS = NS * K               # 100000 matmul columns per core
FSH = NSH * K             # 50000 columns per half-stream
CHUNK_NODES = 256         # nodes per DMA chunk per stream ([128,4096]=2MB)
MM_NODES = 32             # nodes per matmul (32*16 = 512 = max fp32 free)

_CACHE = {}


def _chunk_schedule():
    """Per-half-stream chunk sizes.  Small leading chunks let the first
    bytes land before the full-chunk descriptor generation would; the
    tapered tail keeps the post-last-DMA compute drain to one tiny
    chunk's worth of K-max/relu/flush."""
    head = [64, 192]
    tail = [128, 64, 64, 32, 21]
    chunks = list(head)
    rem = NSH - sum(head) - sum(tail)
    while rem > 0:
        c = min(CHUNK_NODES, rem)
        chunks.append(c)
        rem -= c
    chunks += tail
    assert sum(chunks) == NSH
    return chunks


def _build():
    if "nc" in _CACHE:
        return _CACHE["nc"]
    import concourse.bacc as bacc
    import concourse.mybir as mybir
    from concourse.tile import TileContext

    fp32 = mybir.dt.float32
    nc = bacc.Bacc(
        "TRN2", target_bir_lowering=False, debug=False, num_devices=NCORES
    )
    x = nc.dram_tensor("x", [2 * C, FS], fp32, kind="ExternalInput")
    cw = nc.dram_tensor("cw", [2 * C, COUT + 1], fp32, kind="ExternalInput")
    y = nc.dram_tensor("y", [COUT, NS], fp32, kind="ExternalOutput")

    chunks = _chunk_schedule()
    n_chunks = len(chunks)

    with TileContext(nc) as tc:
        with (
            tc.tile_pool(name="const", bufs=1) as cpool,
            tc.tile_pool(name="xa", bufs=5) as xapool,
            tc.tile_pool(name="xb", bufs=5) as xbpool,
            tc.tile_pool(name="psum", bufs=8, space="PSUM") as ppool,
            tc.tile_pool(name="rt", bufs=4) as opool,
        ):
            ct = cpool.tile([2 * C, COUT + 1], fp32)
            wt = ct[:, 0:COUT]
            bt = ct[:, COUT : COUT + 1]

            # output tiles are shared by REGIONS of consecutive chunks:
            # one bias+relu and one flush pair per region.  Region rows in
            # DRAM are then >=1.75KB per channel (vs 1KB for per-chunk
            # flushes), well past the DMA descriptor-efficiency knee, and
            # the flush load on the slowest SDMA engine drops with it.
            region_of = {}      # ci -> region index
            regions = []        # (ci_first, ci_last, node0, nodes)
            ci0 = node0 = 0
            for want in (512, 1024, 1024, 448, None):
                ci1, nodes = ci0, 0
                while ci1 < n_chunks and (want is None or nodes < want):
                    nodes += chunks[ci1]
                    ci1 += 1
                for ci in range(ci0, ci1):
                    region_of[ci] = len(regions)
                regions.append((ci0, ci1 - 1, node0, nodes))
                node0 += nodes
                ci0 = ci1
            rtiles = {}

            def flush(ri):
                """Write region ri's relu'd tile out on the HWDGE rings."""
                rt = rtiles.pop(ri)
                _, _, node0, nodes = regions[ri]
                nc.sync.dma_start(y[:, node0 : node0 + nodes], rt[0:COUT, :nodes])
                nc.scalar.dma_start(
                    y[:, NSH + node0 : NSH + node0 + nodes],
                    rt[COUT : 2 * COUT, :nodes],
                )

            node = 0  # offset within the half-stream
            for ci, nn_ in enumerate(chunks):
                cols = nn_ * K
                xta = xapool.tile([2 * C, CHUNK_NODES * K], fp32, tag="xa")
                xtb = xbpool.tile([2 * C, CHUNK_NODES * K], fp32, tag="xb")
                if ci == 0:
                    # constants on the (otherwise idle) SWDGE queue: their
                    # 128 tiny descriptors would stall a HWDGE ring's
                    # descriptor generation ahead of the first input chunk
                    nc.gpsimd.dma_start(ct[:], cw[:])
                    # first chunk split across both rings so descriptor
                    # generation for the first bytes runs in parallel
                    nc.sync.dma_start(xta[0:C, :cols], x[0:C, 0:cols])
                    nc.scalar.dma_start(
                        xta[C : 2 * C, :cols], x[C : 2 * C, 0:cols]
                    )
                    nc.scalar.dma_start(xtb[0:C, :cols], x[0:C, FSH : FSH + cols])
                    nc.sync.dma_start(
                        xtb[C : 2 * C, :cols], x[C : 2 * C, FSH : FSH + cols]
                    )
                else:
                    nc.sync.dma_start(
                        xta[:, :cols], x[:, node * K : node * K + cols]
                    )
                    nc.scalar.dma_start(
                        xtb[:, :cols],
                        x[:, FSH + node * K : FSH + node * K + cols],
                    )
                ri = region_of[ci]
                ci_first, ci_last, rnode0, rnodes = regions[ri]
                if ci == ci_first:
                    rt_new = opool.tile([2 * C, 1024], fp32, tag="rt")
                    rtiles[ri] = rt_new
                rt = rtiles[ri]
                off = node - rnode0
                ngroups = (nn_ + MM_NODES - 1) // MM_NODES
                for t in range(ngroups):
                    g0 = t * MM_NODES
                    gn = min(MM_NODES, nn_ - g0)
                    ps = ppool.tile([2 * C, MM_NODES * K], fp32, tag="ps")
                    nc.tensor.matmul(
                        ps[0:COUT, : gn * K],
                        wt,
                        xta[:, g0 * K : (g0 + gn) * K],
                        start=True,
                        stop=True,
                    )
                    nc.tensor.matmul(
                        ps[COUT : 2 * COUT, : gn * K],
                        wt,
                        xtb[:, g0 * K : (g0 + gn) * K],
                        start=True,
                        stop=True,
                    )
                    nc.vector.tensor_reduce(
                        rt[:, off + g0 : off + g0 + gn],
                        ps[:, : gn * K].rearrange("p (n k) -> p n k", k=K),
                        axis=mybir.AxisListType.X,
                        op=mybir.AluOpType.max,
                    )
                if ci == ci_last:
                    # bias+relu fused on DVE over the whole region: runs
                    # in-order right after the last K-max, no cross-engine
                    # sem hop, no ACT-table load on the dispatch path
                    nc.vector.tensor_scalar(
                        rt[:, :rnodes],
                        rt[:, :rnodes],
                        bt,
                        0.0,
                        mybir.AluOpType.add,
                        mybir.AluOpType.max,
                    )
                # flush a region whose relu is guaranteed long done, so
                # the flush's wait never parks the ring's sequencer and
                # blocks later input dispatches
                for pri in sorted(rtiles):
                    if regions[pri][1] <= ci - 2:
                        flush(pri)
                node += nn_
            for ri in sorted(rtiles):
                flush(ri)

    nc.compile()
    _CACHE["nc"] = nc
    return nc


def _prep_inputs(x_i, x_j, W, b):
    x_i = np.asarray(x_i, dtype=np.float32).reshape(C, N * K)
    x_j = np.asarray(x_j, dtype=np.float32).reshape(C, N * K)
    W = np.asarray(W, dtype=np.float32)
    b = np.asarray(b, dtype=np.float32)

    W1, W2 = W[:, :C], W[:, C:]
    wT = np.concatenate([(W1 - W2).T, W2.T], axis=0)  # [2C, COUT]
    bias = np.concatenate([b, b]).reshape(2 * C, 1)  # both partition halves
    cw = np.ascontiguousarray(np.concatenate([wT, bias], axis=1))

    xfull = np.empty((NCORES, 2 * C, FS), dtype=np.float32)
    for s in range(NCORES):
        xfull[s, :C] = x_i[:, s * FS : (s + 1) * FS]
        xfull[s, C:] = x_j[:, s * FS : (s + 1) * FS]

    return [{"x": xfull[s], "cw": cw} for s in range(NCORES)]


def run(x_i, x_j, W, b, **spmd_kwargs):
    """Build + run, returning (full_output, BassKernelResults)."""
    from concourse.bass_utils import run_bass_kernel_spmd

    nc = _build()
    in_maps = _prep_inputs(x_i, x_j, W, b)
    res = run_bass_kernel_spmd(nc, in_maps, list(range(NCORES)), **spmd_kwargs)
    y = np.concatenate(
        [res.results[s]["y"] for s in range(NCORES)], axis=1
    )  # [COUT, N]
    return y.reshape(B, COUT, N, 1), res


def kernel(x_i, x_j, W, b):
    out, _ = run(x_i, x_j, W, b)
    return out
